# revision 1
# baseline (speedup 1.0000x reference)
"""Trainium2 Bass kernel for a WaveNet-style dilated-conv stack.

Network (per reference):
  x1 = conv1d(x, Wc, bc, d=1, pad=1)                      # 1 -> 32, host-side (exact fp32)
  for l in 27 layers, d = 2^(l%9):
      g = tanh(conv(x, Wt_l, d)) * sigmoid(conv(x, Ws_l, d))   # 32->32, k=3, pad=d
      skip += conv1x1(g, Wskip_l)                              # 32->512
      x = conv1x1(g, Wdense_l) + x
  out = conv1x1(relu(conv1x1(skip, Wp1)), Wp2)            # 512->512->256
  return log_softmax(out, axis=channels)

Device strategy (8 cores, sequence-parallel):
  - Wp1 folded into skip weights on host: W1s_l = Wp1 @ Wskip_l, so
    h = Wp1@skip + bp1 is accumulated directly (512 ch), then relu -> Wp2.
  - Each core owns 16384 steps, processed as 2 halves of 8192 with a 1536-step
    halo (total receptive radius of the dilated stack is 1533).  Edge windows
    use zero/stale padding; contamination moves <= d per layer, so the valid
    region stays exact.  No cross-core communication.
  - g for 4 consecutive layers is staged in a [128, W] "ring" so the skip
    projection runs as single K=128 matmuls.  The dense 1x1 conv is a single
    K=128 matmul with zeros outside the layer's ring strip (this backend
    rejects PSUM accumulation across different PE row strips).
  - bf16 matmuls (fp32 PSUM), fp32 post-processing.
"""

import numpy as np
import ml_dtypes

BF16 = ml_dtypes.bfloat16

DIL = [2 ** i for i in range(9)] * 3
L = len(DIL)            # 27
RD, SD, QD = 32, 512, 256
T = 131072
NCORES = 8
V = T // NCORES         # 16384 per core
VH = V // 2             # 8192 per half
HALO = 1536             # >= 1533 total dilation radius
PAD = 256               # >= max dilation, so tap reads never go OOB
WH = VH + 2 * HALO      # 11264 computed window per half
WA = WH + 2 * PAD       # 11776 allocated width per half
VOFF = HALO + PAD       # 1792 valid-region offset inside the window
NGRP = (L + 3) // 4     # 7 groups of (up to) 4 layers for K=128 skip matmuls
ATILE = 1024            # activation tile width
NA = WH // ATILE        # 11 act tiles per layer per half
NB = VH // 512          # 16 valid 512-col blocks per half

_cache = {}
_last_run = {}


def _build():
    from contextlib import ExitStack

    import concourse.bacc as bacc
    import concourse.mybir as mybir
    import concourse.tile as tile

    dt = mybir.dt
    AF = mybir.ActivationFunctionType
    ALU = mybir.AluOpType
    f32, bf16 = dt.float32, dt.bfloat16

    nc = bacc.Bacc("TRN2", target_bir_lowering=False, debug=False,
                   num_devices=NCORES)

    def din(name, shape, dty):
        return nc.dram_tensor(name, shape, dty, kind="ExternalInput").ap()

    xin_d = din("xin", [RD, 2 * WA], bf16)
    wg_d = din("wg", [64, L * 3 * 64], bf16)       # gated lhsT, 2 strip replicas
    wdx_d = din("wdx", [128, L * RD], bf16)        # dense lhsT (strip-embedded)
    idw_d = din("idw", [128, 2 * RD], bf16)        # residual identity lhsT
    wskp_d = din("wskp", [128, NGRP * 4 * 128], bf16)  # skip lhsT per (grp, m)
    wp2_d = din("wp2", [128, 8 * 128], bf16)       # Wp2 lhsT per (q, p)
    bts_d = din("bts", [RD, L], f32)
    bss_d = din("bss", [RD, L], f32)
    bdc_d = din("bdc", [RD, L], f32)
    hb_d = din("hb", [128, 4], f32)
    bp2c_d = din("bp2c", [128, 2], f32)
    sumw_d = din("sumw", [128, 1], f32)
    nones_d = din("nones", [1, 128], f32)
    out_d = nc.dram_tensor("out", [QD, V], f32, kind="ExternalOutput").ap()

    with tile.TileContext(nc) as tc, ExitStack() as top:
        wp = top.enter_context(tc.tile_pool(name="wp", bufs=1))

        def load(d, tag):
            t = wp.tile(list(d.shape), d.dtype, tag=tag, name=tag)
            nc.sync.dma_start(t[:], d[:])
            return t

        wg = load(wg_d, "wg")
        wdx = load(wdx_d, "wdx")
        idw = load(idw_d, "idw")
        wskp = load(wskp_d, "wskp")
        wp2 = load(wp2_d, "wp2")
        bts = load(bts_d, "bts")
        bss = load(bss_d, "bss")
        bdc = load(bdc_d, "bdc")
        hb = load(hb_d, "hb")
        bp2c = load(bp2c_d, "bp2c")
        sumw = load(sumw_d, "sumw")
        nones = load(nones_d, "nones")

        # x ping-pongs between partition strips 0/1 of one [128, W] tensor so
        # the residual add runs on the PE as a K=128 identity matmul.
        xx = wp.tile([128, WA], bf16, tag="xx", name="xx")
        ring = wp.tile([128, WA], bf16, tag="ring", name="ring")
        h = wp.tile([128, 4 * VH], bf16, tag="h", name="h")
        nc.vector.memset(xx[:], 0.0)
        nc.vector.memset(ring[:], 0.0)

        for half in range(2):
            nc.sync.dma_start(xx[0:RD, :], xin_d[:, half * WA:(half + 1) * WA])
            with ExitStack() as lctx:
                pg = lctx.enter_context(
                    tc.tile_pool(name=f"pg{half}", bufs=3, space="PSUM"))
                pk = lctx.enter_context(
                    tc.tile_pool(name=f"pk{half}", bufs=2, space="PSUM"))
                tu = lctx.enter_context(tc.tile_pool(name=f"tu{half}", bufs=3))

                for l in range(L):
                    d = DIL[l]
                    j = l % 4
                    G = l // 4
                    sc = RD * (l % 2)        # strip of x_l
                    sn = RD * ((l + 1) % 2)  # strip of x_{l+1}
                    for a in range(NA):
                        b0 = PAD + a * ATILE
                        pgt = pg.tile([128, ATILE], f32, tag="pg", name="pg")
                        for s in range(2):
                            c0 = b0 + s * 512
                            for k in range(3):
                                nc.tensor.matmul(
                                    pgt[0:64, s * 512:(s + 1) * 512],
                                    wg[sc:sc + RD,
                                       (l * 3 + k) * 64:(l * 3 + k + 1) * 64],
                                    xx[sc:sc + RD, c0 + (k - 1) * d:
                                       c0 + (k - 1) * d + 512],
                                    start=(k == 0), stop=(k == 2),
                                    tile_position=(sc, 0))
                        tt = tu.tile([RD, ATILE], bf16, tag="t", name="t")
                        uu = tu.tile([RD, ATILE], bf16, tag="u", name="u")
                        nc.scalar.activation(tt[:], pgt[0:RD, :], AF.Tanh,
                                             bias=bts[:, l:l + 1])
                        nc.scalar.activation(uu[:], pgt[RD:2 * RD, :],
                                             AF.Sigmoid, bias=bss[:, l:l + 1])
                        nc.vector.tensor_mul(
                            ring[RD * j:RD * (j + 1), b0:b0 + ATILE],
                            tt[:], uu[:])
                        for s in range(2):
                            c0 = b0 + s * 512
                            pxs = pgt[64 + RD * s:96 + RD * s,
                                      s * 512:(s + 1) * 512]
                            nc.tensor.matmul(
                                pxs, wdx[:, l * RD:(l + 1) * RD],
                                ring[:, c0:c0 + 512], start=True, stop=False,
                                tile_position=(0, 64 + RD * s))
                            nc.tensor.matmul(
                                pxs, idw[:, RD * (l % 2):RD * (l % 2) + RD],
                                xx[:, c0:c0 + 512], start=False, stop=True,
                                tile_position=(0, 64 + RD * s))
                            # x_new = psum + bdense (residual already in psum)
                            nc.vector.tensor_scalar_add(
                                xx[sn:sn + RD, c0:c0 + 512], pxs,
                                bdc[:, l:l + 1])

                    if j == 3 or l == L - 1:
                        # skip contribution of this 4-layer group (K=128)
                        for m in range(4):
                            for cb in range(NB):
                                c0 = VOFF + cb * 512
                                pst = pk.tile([128, 512], f32, tag="pk",
                                              name="pk")
                                nc.tensor.matmul(
                                    pst[:],
                                    wskp[:, (G * 4 + m) * 128:
                                         (G * 4 + m + 1) * 128],
                                    ring[:, c0:c0 + 512],
                                    start=True, stop=True)
                                hcol = m * VH + cb * 512
                                if G == 0:
                                    nc.vector.tensor_copy(
                                        h[:, hcol:hcol + 512], pst[:])
                                else:
                                    nc.vector.tensor_add(
                                        h[:, hcol:hcol + 512],
                                        h[:, hcol:hcol + 512], pst[:])

            with ExitStack() as pctx:
                pop = pctx.enter_context(
                    tc.tile_pool(name=f"po{half}", bufs=4, space="PSUM"))
                psp = pctx.enter_context(
                    tc.tile_pool(name=f"ps{half}", bufs=2, space="PSUM"))
                pqp = pctx.enter_context(
                    tc.tile_pool(name=f"pq{half}", bufs=2, space="PSUM"))
                sp = pctx.enter_context(tc.tile_pool(name=f"sp{half}", bufs=2))
                for cb in range(NB):
                    rr = sp.tile([128, 4 * 512], bf16, tag="r", name="r")
                    for m in range(4):
                        nc.scalar.activation(
                            rr[:, m * 512:(m + 1) * 512],
                            h[:, m * VH + cb * 512:m * VH + cb * 512 + 512],
                            AF.Relu, bias=hb[:, m:m + 1])
                    pos = []
                    for p in range(2):
                        pot = pop.tile([128, 512], f32, tag="po", name="po")
                        for q in range(4):
                            nc.tensor.matmul(
                                pot[:],
                                wp2[:, (q * 2 + p) * 128:(q * 2 + p + 1) * 128],
                                rr[:, q * 512:(q + 1) * 512],
                                start=(q == 0), stop=(q == 3))
                        pos.append(pot)
                    ee = sp.tile([128, 1024], f32, tag="e", name="e")
                    for p in range(2):
                        nc.scalar.activation(ee[:, p * 512:(p + 1) * 512],
                                             pos[p][:], AF.Exp,
                                             bias=bp2c[:, p:p + 1])
                    pst = psp.tile([128, 512], f32, tag="ps", name="ps")
                    for p in range(2):
                        nc.tensor.matmul(pst[0:1, :], sumw[:],
                                         ee[:, p * 512:(p + 1) * 512],
                                         start=(p == 0), stop=(p == 1))
                    lss = sp.tile([1, 512], f32, tag="ls", name="ls")
                    nc.scalar.activation(lss[:], pst[0:1, :], AF.Ln)
                    pqt = pqp.tile([128, 512], f32, tag="pq", name="pq")
                    nc.tensor.matmul(pqt[:], nones[:], lss[:],
                                     start=True, stop=True)
                    oo = sp.tile([128, 1024], f32, tag="o", name="o")
                    oo2 = sp.tile([128, 1024], f32, tag="o2", name="o2")
                    for p in range(2):
                        nc.scalar.activation(oo[:, p * 512:(p + 1) * 512],
                                             pos[p][:], AF.Identity,
                                             bias=bp2c[:, p:p + 1])
                        nc.vector.tensor_add(oo2[:, p * 512:(p + 1) * 512],
                                             oo[:, p * 512:(p + 1) * 512],
                                             pqt[:])
                        c0 = half * VH + cb * 512
                        nc.sync.dma_start(
                            out_d[p * 128:(p + 1) * 128, c0:c0 + 512],
                            oo2[:, p * 512:(p + 1) * 512])

    nc.compile()
    return nc


def _prep_host(inputs):
    """Host-side exact fp32 preprocessing: initial conv, weight packing."""
    x = np.asarray(inputs["x"], np.float32)
    Wc = np.asarray(inputs["Wc"], np.float32)
    bc = np.asarray(inputs["bc"], np.float32)
    Wt = np.asarray(inputs["Wt"], np.float32)
    bt = np.asarray(inputs["bt"], np.float32)
    Ws = np.asarray(inputs["Ws"], np.float32)
    bs = np.asarray(inputs["bs"], np.float32)
    Wskip = np.asarray(inputs["Wskip"], np.float32)
    bskip = np.asarray(inputs["bskip"], np.float32)
    Wdense = np.asarray(inputs["Wdense"], np.float32)
    bdense = np.asarray(inputs["bdense"], np.float32)
    Wp1 = np.asarray(inputs["Wp1"], np.float32)
    bp1 = np.asarray(inputs["bp1"], np.float32)
    Wp2 = np.asarray(inputs["Wp2"], np.float32)
    bp2 = np.asarray(inputs["bp2"], np.float32)

    # initial conv (1 -> 32, k=3, pad=1), exact fp32 on host
    x0 = x[0, 0]
    xp = np.pad(x0, (1, 1))
    x1 = (Wc[:, 0, 0:1] * xp[None, 0:T]
          + Wc[:, 0, 1:2] * xp[None, 1:T + 1]
          + Wc[:, 0, 2:3] * xp[None, 2:T + 2]) + bc[:, None]
    xg = np.pad(x1, ((0, 0), (VOFF, VOFF)))

    xin = np.empty((NCORES, RD, 2 * WA), BF16)
    for c in range(NCORES):
        for hf in range(2):
            s = c * V + hf * VH
            xin[c, :, hf * WA:(hf + 1) * WA] = xg[:, s:s + WA].astype(BF16)

    wg = np.zeros((64, L * 3 * 64), np.float32)
    wdx = np.zeros((128, L * RD), np.float32)
    for l in range(L):
        for k in range(3):
            blk = np.concatenate([Wt[l, :, :, k].T, Ws[l, :, :, k].T], axis=1)
            for p in range(2):
                wg[RD * p:RD * (p + 1),
                   (l * 3 + k) * 64:(l * 3 + k + 1) * 64] = blk
        j = l % 4
        wdx[RD * j:RD * (j + 1), l * RD:(l + 1) * RD] = Wdense[l, :, :, 0].T

    idw = np.zeros((128, 2 * RD), np.float32)
    for p in range(2):
        idw[RD * p:RD * (p + 1), RD * p:RD * (p + 1)] = np.eye(RD)

    W1s = np.einsum("ab,lbc->lac", Wp1[:, :, 0], Wskip[:, :, :, 0])  # [L,512,32]
    wskp = np.zeros((128, NGRP * 4 * 128), np.float32)
    for G in range(NGRP):
        for m in range(4):
            for j in range(4):
                l = G * 4 + j
                if l < L:
                    wskp[32 * j:32 * (j + 1),
                         (G * 4 + m) * 128:(G * 4 + m + 1) * 128] = \
                        W1s[l, 128 * m:128 * (m + 1), :].T

    wp2 = np.zeros((128, 8 * 128), np.float32)
    for q in range(4):
        for p in range(2):
            wp2[:, (q * 2 + p) * 128:(q * 2 + p + 1) * 128] = \
                Wp2[128 * p:128 * (p + 1), 128 * q:128 * (q + 1), 0].T

    hbias = Wp1[:, :, 0] @ bskip.sum(axis=0) + bp1     # [512]
    hb = hbias.reshape(4, 128).T.copy()                # [128, 4]

    shared = {
        "wg": wg.astype(BF16),
        "wdx": wdx.astype(BF16),
        "idw": idw.astype(BF16),
        "wskp": wskp.astype(BF16),
        "wp2": wp2.astype(BF16),
        "bts": np.ascontiguousarray(bt.T.astype(np.float32)),
        "bss": np.ascontiguousarray(bs.T.astype(np.float32)),
        "bdc": np.ascontiguousarray(bdense.T.astype(np.float32)),
        "hb": np.ascontiguousarray(hb.astype(np.float32)),
        "bp2c": np.ascontiguousarray(bp2.reshape(2, 128).T.astype(np.float32)),
        "sumw": np.ones((128, 1), np.float32),
        "nones": np.full((1, 128), -1.0, np.float32),
    }
    return xin, shared


def kernel(**inputs):
    from concourse.bass_utils import run_bass_kernel_spmd

    xin, shared = _prep_host(inputs)
    if "nc" not in _cache:
        _cache["nc"] = _build()
    nc = _cache["nc"]

    in_maps = [dict(shared, xin=np.ascontiguousarray(xin[c]))
               for c in range(NCORES)]
    res = run_bass_kernel_spmd(nc, in_maps, core_ids=list(range(NCORES)))

    _last_run["nc"] = nc
    _last_run["in_maps"] = in_maps

    out = np.empty((1, QD, T), np.float32)
    for c in range(NCORES):
        out[0, :, c * V:(c + 1) * V] = res.results[c]["out"]
    return out



# revision 2
# speedup vs baseline: 1.0718x; 1.0718x over previous
"""Trainium2 Bass kernel v2 for the WaveNet-style dilated-conv stack.

Design vs baseline:
  - Single pass per core (V=16384) with exact per-layer shrinking margins
    (M_l = sum of remaining dilations) instead of halves + fixed halo.
  - Gated conv = ONE K=96 bf16 matmul per 1024-col pair: x plus two
    DMA-maintained shifted replicas (x<<d, x<<2d) at partitions 0:96.
  - PSUM quad packing: two block-pairs per [128,1024] bank, pair 0 rows
    [tanh;sig], pair 1 rows [sig;tanh] so sigmoid spans contiguous
    partitions 32:96 -> one [64,1024] activation instr per 4 blocks.
  - Residual bias-free (dense biases folded into later gated biases on
    host); residual add on DVE/Pool alternating, no identity matmul.
  - ring (gate activations g) in fp8e4: dense 1x1 plain fp8, skip
    projection fp8 DoubleRow K=256 (8-layer groups) -> 4x fewer skip
    matmuls and half the h-accumulate traffic.
  - h (= Wp1@skip + bias) accumulated in fp8 SBUF [128, 4*V].
  - log_softmax tail as baseline (exp/sum-matmul/ln/broadcast matmul).
"""

import numpy as np
import ml_dtypes

BF16 = ml_dtypes.bfloat16
F8 = ml_dtypes.float8_e4m3fn

DIL = [2 ** i for i in range(9)] * 3
L = len(DIL)                      # 27
RD, SD, QD = 32, 512, 256
T = 131072
NCORES = 8
V = T // NCORES                   # 16384
VOFF = 1792
W = 20480                         # xt width; region+tap reads stay in bounds
NG = (L + 7) // 8                 # 4 groups of (up to) 8 layers
NB = V // 512                     # 32 valid blocks

# suffix dilation sums: S[l] = sum(DIL[l:]) ; margins M[l] = S[l+1]
S = [0] * (L + 1)
for _l in range(L - 1, -1, -1):
    S[_l] = S[_l + 1] + DIL[_l]
MARG = [S[_l + 1] for _l in range(L)]
NBLK = [-(-(V + 2 * MARG[_l]) // 512) for _l in range(L)]
START = [VOFF - MARG[_l] for _l in range(L)]

_cache = {}
_last_run = {}


def _build(hb_zero=False):
    from contextlib import ExitStack

    import concourse.bacc as bacc
    import concourse.mybir as mybir
    import concourse.tile as tile

    dt = mybir.dt
    AF = mybir.ActivationFunctionType
    PM = mybir.MatmulPerfMode
    ALU = mybir.AluOpType
    f32, bf16, f8 = dt.float32, dt.bfloat16, dt.float8e4

    nc = bacc.Bacc("TRN2", target_bir_lowering=False, debug=False,
                   num_devices=NCORES)

    def din(name, shape, dty):
        return nc.dram_tensor(name, shape, dty, kind="ExternalInput").ap()

    xin_d = din("xin", [RD, W], bf16)
    # gated lhsT per layer [96, 64]: cols 0:32 tanh, 32:64 sigmoid(z/2)
    wgt_d = din("wgt", [96, L * 64], bf16)
    wdx_d = din("wdx", [128, L * 64], f8)         # dense lhsT, strip-placed
    wskp_d = din("wskp", [128, NG * 4 * 256], f8)  # skip lhsT DR per (G,m)
    wp2_d = din("wp2", [128, 2 * 2 * 256], f8)    # Wp2 lhsT DR per (p,qp)
    bgt_d = din("bgt", [128, L], f32)   # gated bias rows [bt, bs/2]x2
    hb_d = din("hb", [128, 4], f32)
    bp2c_d = din("bp2c", [128, 2], f32)
    sumw_d = din("sumw", [128, 1], bf16)
    nones_d = din("nones", [1, 128], bf16)
    out_d = nc.dram_tensor("out", [QD, V], f32, kind="ExternalOutput").ap()

    with tile.TileContext(nc) as tc, ExitStack() as top:
        wp = top.enter_context(tc.tile_pool(name="wp", bufs=1))

        def load(d, tag):
            t = wp.tile(list(d.shape), d.dtype, tag=tag, name=tag)
            nc.sync.dma_start(t[:], d[:])
            return t

        wgt = load(wgt_d, "wgt")
        wdx = load(wdx_d, "wdx")
        wskp = load(wskp_d, "wskp")
        wp2 = load(wp2_d, "wp2")
        bgt = load(bgt_d, "bgt")
        hb = load(hb_d, "hb")
        bp2c = load(bp2c_d, "bp2c")
        sumw = load(sumw_d, "sumw")
        nones = load(nones_d, "nones")

        xt = wp.tile([96, W], bf16, tag="xt", name="xt")
        ring = wp.tile([128, 2 * W], f8, tag="ring", name="ring")
        h = wp.tile([128, 4 * V], f8, tag="h", name="h")

        wdxv = wdx.rearrange("p (l two m) -> p l two m", two=2, m=RD)
        wskpv = wskp.rearrange("p (g two m) -> p g two m", two=2, m=128)
        wp2v = wp2.rearrange("p (q two m) -> p q two m", two=2, m=128)
        rv = ring.rearrange("p (two n) -> p two n", two=2)

        # input + initial replicas (layer 0 has d=1)
        nc.sync.dma_start(xt[0:RD, :], xin_d[:])
        d0 = DIL[0]
        for half in range(2):
            cs = half * (W // 2)
            ce = min(W - 2 * d0, (half + 1) * (W // 2))
            nc.sync.dma_start(xt[32:64, cs:ce], xt[0:RD, cs + d0:ce + d0])
            nc.sync.dma_start(xt[64:96, cs:ce],
                              xt[0:RD, cs + 2 * d0:ce + 2 * d0])

        with ExitStack() as lctx:
            pgp = lctx.enter_context(
                tc.tile_pool(name="pgp", bufs=2, space="PSUM"))
            pdp = lctx.enter_context(
                tc.tile_pool(name="pdp", bufs=2, space="PSUM"))
            tu = lctx.enter_context(tc.tile_pool(name="tu", bufs=3))

            for l in range(L):
                d = DIL[l]
                j = l % 8
                G = l // 8
                a = j % 4
                kt = j // 4
                nb = NBLK[l]
                st = START[l]

                # ---- gated convs: quads of up to 4 blocks ----
                # pg quadrants: b0=(rows 0:64 [T;S], cols 0:512),
                # b1=(rows 0:64, cols 512:1024), b2=(rows 64:128 [S;T],
                # cols 0:512), b3=(rows 64:128, cols 512:1024).  sigmoid
                # then spans rows 32:96 contiguously for all 4 blocks.
                for q in range(-(-nb // 4)):
                    qn = min(4, nb - 4 * q)
                    topn = min(2, qn)
                    botn = qn - topn
                    ctop = st + 4 * q * 512
                    cbot = ctop + 1024
                    pg = pgp.tile([128, 1024], f32, tag="pg", name="pg")
                    for i in range(qn):
                        e = 0 if i < 2 else 1      # row half
                        col = (i % 2) * 512
                        cc = (ctop if e == 0 else cbot) + (i % 2) * 512
                        nc.tensor.matmul(
                            pg[64 * e:64 * e + 64, col:col + 512],
                            wgt[:, l * 64:l * 64 + 64],
                            xt[0:96, cc - d:cc - d + 512],
                            start=True, stop=True,
                            tile_position=(0, 64 * e))
                    tt = tu.tile([128, 1024], bf16, tag="tt", name="tt")
                    ss = tu.tile([128, 1024], bf16, tag="ss", name="ss")
                    nt = 512 * topn
                    nb_ = 512 * botn
                    rows = 128 if botn else 64
                    # single tanh: rows [t; tanh(z/2)] per pair; sigmoid is
                    # (1+s)/2, the x2 folded into halved dense/skip weights
                    nc.scalar.activation(tt[0:rows, 0:1024],
                                         pg[0:rows, 0:1024], AF.Tanh,
                                         bias=bgt[0:rows, l:l + 1])
                    nc.scalar.activation(ss[0:32, 0:nt], tt[32:64, 0:nt],
                                         AF.Identity, bias=1.0)
                    nc.gpsimd.tensor_mul(
                        rv[RD * a:RD * a + RD, kt, ctop:ctop + nt],
                        ss[0:32, 0:nt], tt[0:32, 0:nt])
                    if botn:
                        nc.scalar.activation(ss[64:96, 0:nb_],
                                             tt[96:128, 0:nb_],
                                             AF.Identity, bias=1.0)
                        nc.gpsimd.tensor_mul(
                            rv[RD * a:RD * a + RD, kt, cbot:cbot + nb_],
                            ss[64:96, 0:nb_], tt[64:96, 0:nb_])

                if l < L - 1:
                    # ---- dense 1x1 (plain fp8) + residual ----
                    for p in range(-(-nb // 2)):
                        b0 = 2 * p
                        npr = 2 if 2 * p + 1 < nb else 1
                        cw = 512 * npr
                        c0 = st + b0 * 512
                        pd = pdp.tile([RD, 1024], f32, tag="pd", name="pd")
                        for s in range(npr):
                            cs = c0 + 512 * s
                            nc.tensor.matmul(
                                pd[:, 512 * s:512 * s + 512],
                                wdxv[RD * a:RD * a + RD, l, kt],
                                rv[RD * a:RD * a + RD, kt, cs:cs + 512],
                                start=True, stop=True,
                                tile_position=(RD * a, 0))
                        nc.vector.tensor_add(xt[0:RD, c0:c0 + cw],
                                             pd[:, 0:cw],
                                             xt[0:RD, c0:c0 + cw])

                    # ---- refresh tap replicas for layer l+1 ----
                    dn = DIL[l + 1]
                    st2 = START[l + 1]
                    wd2 = NBLK[l + 1] * 512
                    nch = 4
                    chw = -(-wd2 // nch)
                    for ch in range(nch):
                        cs = st2 - dn + ch * chw
                        ce = min(st2 - dn + (ch + 1) * chw, st2 + wd2)
                        if ce <= cs:
                            continue
                        nc.sync.dma_start(xt[32:64, cs:ce],
                                          xt[0:RD, cs + dn:ce + dn])
                        nc.sync.dma_start(xt[64:96, cs:ce],
                                          xt[0:RD, cs + 2 * dn:ce + 2 * dn])

                # ---- skip projection at group end (fp8 DR, K=256) ----
                if j == 7 or l == L - 1:
                    for cp in range(NB // 2):
                        c0 = VOFF + cp * 1024
                        for m in range(4):
                            pk = pdp.tile([128, 1024], f32, tag="pd",
                                          name="pk")
                            for s2 in range(2):
                                nc.tensor.matmul(
                                    pk[:, 512 * s2:512 * s2 + 512],
                                    wskpv[:, G * 4 + m],
                                    rv[:, :, c0 + 512 * s2:c0 + 512 * s2 + 512],
                                    start=True, stop=True,
                                    perf_mode=PM.DoubleRow)
                            hs = h[:, m * V + cp * 1024:
                                   m * V + cp * 1024 + 1024]
                            if G == 0:
                                nc.vector.tensor_copy(hs, pk[:])
                            else:
                                nc.vector.tensor_add(hs, pk[:], hs)

        # ---- post stage: relu -> Wp2 -> log_softmax -> out ----
        with ExitStack() as pctx:
            pop = pctx.enter_context(
                tc.tile_pool(name="pop", bufs=4, space="PSUM"))
            psp = pctx.enter_context(
                tc.tile_pool(name="psp", bufs=2, space="PSUM"))
            sp = pctx.enter_context(tc.tile_pool(name="sp", bufs=2))
            ALU = mybir.AluOpType
            for cb in range(NB):
                c0 = cb * 512
                rr = sp.tile([128, 4 * 512], f8, tag="rr", name="rr")
                rrv = rr.rearrange("p (q n) -> p q n", q=4)
                if hb_zero:
                    hv = h.rearrange("p (q n) -> p q n", q=4)
                    nc.scalar.activation(rrv[:, :, :], hv[:, :, c0:c0 + 512],
                                         AF.Relu)
                else:
                    for m in range(4):
                        nc.scalar.activation(
                            rr[:, m * 512:(m + 1) * 512],
                            h[:, m * V + c0:m * V + c0 + 512],
                            AF.Relu, bias=hb[:, m:m + 1])
                ee = sp.tile([128, 1024], bf16, tag="ee", name="ee")
                pos = []
                for pp_ in range(2):
                    po = pop.tile([128, 512], f32, tag="po", name="po")
                    nc.tensor.matmul(po[:], wp2v[:, 2 * pp_], rrv[:, 0:2, :],
                                     start=True, stop=False,
                                     perf_mode=PM.DoubleRow)
                    nc.tensor.matmul(po[:], wp2v[:, 2 * pp_ + 1],
                                     rrv[:, 2:4, :],
                                     start=False, stop=True,
                                     perf_mode=PM.DoubleRow)
                    nc.scalar.activation(ee[:, 512 * pp_:512 * pp_ + 512],
                                         po[:], AF.Exp,
                                         bias=bp2c[:, pp_:pp_ + 1])
                    pos.append(po)
                ps_ = psp.tile([1, 512], f32, tag="ps", name="ps")
                nc.tensor.matmul(ps_[0:1, :], sumw[:], ee[:, 0:512],
                                 start=True, stop=False)
                nc.tensor.matmul(ps_[0:1, :], sumw[:], ee[:, 512:1024],
                                 start=False, stop=True)
                lss = sp.tile([1, 512], bf16, tag="ls", name="ls")
                nc.scalar.activation(lss[:], ps_[0:1, :], AF.Ln)
                oo = sp.tile([128, 1024], f32, tag="oo", name="oo")
                for pp_ in range(2):
                    # accumulate -ln(sumexp) into the Wp2 psum, then one
                    # Identity act adds bp2 and writes the final output
                    nc.tensor.matmul(pos[pp_][:], nones[:], lss[:],
                                     start=False, stop=True,
                                     skip_group_check=True)
                    nc.scalar.activation(oo[:, 512 * pp_:512 * pp_ + 512],
                                         pos[pp_][:], AF.Identity,
                                         bias=bp2c[:, pp_:pp_ + 1])
                    nc.sync.dma_start(
                        out_d[pp_ * 128:(pp_ + 1) * 128, c0:c0 + 512],
                        oo[:, 512 * pp_:512 * pp_ + 512])

    nc.compile()
    return nc


def _prep_host(inputs):
    """Host-side fp32 preprocessing: initial conv, bias folding, packing."""
    x = np.asarray(inputs["x"], np.float32)
    Wc = np.asarray(inputs["Wc"], np.float32)
    bc = np.asarray(inputs["bc"], np.float32)
    Wt = np.asarray(inputs["Wt"], np.float32)
    bt = np.asarray(inputs["bt"], np.float32)
    Ws = np.asarray(inputs["Ws"], np.float32)
    bs = np.asarray(inputs["bs"], np.float32)
    Wskip = np.asarray(inputs["Wskip"], np.float32)
    bskip = np.asarray(inputs["bskip"], np.float32)
    Wdense = np.asarray(inputs["Wdense"], np.float32)
    bdense = np.asarray(inputs["bdense"], np.float32)
    Wp1 = np.asarray(inputs["Wp1"], np.float32)
    bp1 = np.asarray(inputs["bp1"], np.float32)
    Wp2 = np.asarray(inputs["Wp2"], np.float32)
    bp2 = np.asarray(inputs["bp2"], np.float32)

    # initial conv (1 -> 32, k=3, pad=1), exact fp32 on host
    x0 = x[0, 0]
    xp = np.pad(x0, (1, 1))
    x1 = (Wc[:, 0, 0:1] * xp[None, 0:T]
          + Wc[:, 0, 1:2] * xp[None, 1:T + 1]
          + Wc[:, 0, 2:3] * xp[None, 2:T + 2]) + bc[:, None]
    xg = np.pad(x1, ((0, 0), (VOFF, W)))  # generous right pad

    xin = np.empty((NCORES, RD, W), BF16)
    for c in range(NCORES):
        s = c * V
        xin[c] = xg[:, s:s + W].astype(BF16)

    # cumulative dense-bias folding
    B = np.zeros((L, RD), np.float32)
    for l in range(1, L):
        B[l] = B[l - 1] + bdense[l - 1]

    wgt = np.zeros((96, L * 64), np.float32)
    btl = np.zeros((L, RD), np.float32)
    bsl = np.zeros((L, RD), np.float32)
    for l in range(L):
        for k in range(3):
            wgt[32 * k:32 * (k + 1), l * 64:l * 64 + 32] = Wt[l, :, :, k].T
            wgt[32 * k:32 * (k + 1),
                l * 64 + 32:l * 64 + 64] = Ws[l, :, :, k].T * 0.5
        btl[l] = bt[l] + Wt[l].sum(axis=2) @ B[l]
        bsl[l] = (bs[l] + Ws[l].sum(axis=2) @ B[l]) * 0.5

    wdx = np.zeros((128, L * 64), np.float32)
    for l in range(L - 1):
        a, kt = (l % 8) % 4, (l % 8) // 4
        wdx[32 * a:32 * a + 32,
            l * 64 + kt * 32:l * 64 + kt * 32 + 32] = Wdense[l, :, :, 0].T * 0.5

    W1s = np.einsum("ab,lbc->lac", Wp1[:, :, 0], Wskip[:, :, :, 0]) * 0.5
    wskp = np.zeros((128, NG * 4 * 256), np.float32)
    for G in range(NG):
        for m in range(4):
            base = (G * 4 + m) * 256
            for j in range(8):
                l = G * 8 + j
                if l < L:
                    a, kt = j % 4, j // 4
                    wskp[32 * a:32 * (a + 1),
                         base + kt * 128:base + kt * 128 + 128] = \
                        W1s[l, 128 * m:128 * (m + 1), :].T

    wp2 = np.zeros((128, 2 * 2 * 256), np.float32)
    for pp_ in range(2):
        for qp in range(2):
            base = (2 * pp_ + qp) * 256
            for half in range(2):
                q = qp * 2 + half
                wp2[:, base + half * 128:base + half * 128 + 128] = \
                    Wp2[128 * pp_:128 * (pp_ + 1),
                        128 * q:128 * (q + 1), 0].T

    hbias = Wp1[:, :, 0] @ bskip.sum(axis=0) + bp1
    hbm = hbias.reshape(4, 128).T.copy()

    shared = {
        "wgt": wgt.astype(BF16),
        "wdx": wdx.astype(F8),
        "wskp": wskp.astype(F8),
        "wp2": wp2.astype(F8),
        "bgt": np.ascontiguousarray(np.concatenate(
            [btl.T, bsl.T, btl.T, bsl.T], axis=0).astype(np.float32)),
        "hb": np.ascontiguousarray(hbm.astype(np.float32)),
        "bp2c": np.ascontiguousarray(bp2.reshape(2, 128).T.astype(np.float32)),
        "sumw": np.ones((128, 1), BF16),
        "nones": np.full((1, 128), -1.0, BF16),
    }
    return xin, shared


def kernel(**inputs):
    from concourse.bass_utils import run_bass_kernel_spmd

    xin, shared = _prep_host(inputs)
    hb_zero = bool(np.all(shared["hb"] == 0.0))
    key = ("nc", hb_zero)
    if key not in _cache:
        _cache[key] = _build(hb_zero)
    nc = _cache[key]

    in_maps = [dict(shared, xin=np.ascontiguousarray(xin[c]))
               for c in range(NCORES)]
    res = run_bass_kernel_spmd(nc, in_maps, core_ids=list(range(NCORES)))

    _last_run["nc"] = nc
    _last_run["in_maps"] = in_maps

    out = np.empty((1, QD, T), np.float32)
    for c in range(NCORES):
        out[0, :, c * V:(c + 1) * V] = res.results[c]["out"]
    return out


# revision 3
# speedup vs baseline: 1.0728x; 1.0010x over previous
"""Trainium2 Bass kernel v2 for the WaveNet-style dilated-conv stack.

Design vs baseline:
  - Single pass per core (V=16384) with exact per-layer shrinking margins
    (M_l = sum of remaining dilations) instead of halves + fixed halo.
  - Gated conv = ONE K=96 bf16 matmul per 1024-col pair: x plus two
    DMA-maintained shifted replicas (x<<d, x<<2d) at partitions 0:96.
  - PSUM quad packing: two block-pairs per [128,1024] bank, pair 0 rows
    [tanh;sig], pair 1 rows [sig;tanh] so sigmoid spans contiguous
    partitions 32:96 -> one [64,1024] activation instr per 4 blocks.
  - Residual bias-free (dense biases folded into later gated biases on
    host); residual add on DVE/Pool alternating, no identity matmul.
  - ring (gate activations g) in fp8e4: dense 1x1 plain fp8, skip
    projection fp8 DoubleRow K=256 (8-layer groups) -> 4x fewer skip
    matmuls and half the h-accumulate traffic.
  - h (= Wp1@skip + bias) accumulated in fp8 SBUF [128, 4*V].
  - log_softmax tail as baseline (exp/sum-matmul/ln/broadcast matmul).
"""

import numpy as np
import ml_dtypes

BF16 = ml_dtypes.bfloat16
F8 = ml_dtypes.float8_e4m3fn

DIL = [2 ** i for i in range(9)] * 3
L = len(DIL)                      # 27
RD, SD, QD = 32, 512, 256
T = 131072
NCORES = 8
V = T // NCORES                   # 16384
VOFF = 1792
W = 20480                         # xt width; region+tap reads stay in bounds
NG = (L + 7) // 8                 # 4 groups of (up to) 8 layers
NB = V // 512                     # 32 valid blocks

# suffix dilation sums: S[l] = sum(DIL[l:]) ; margins M[l] = S[l+1]
S = [0] * (L + 1)
for _l in range(L - 1, -1, -1):
    S[_l] = S[_l + 1] + DIL[_l]
MARG = [S[_l + 1] for _l in range(L)]
NBLK = [-(-(V + 2 * MARG[_l]) // 512) for _l in range(L)]
START = [VOFF - MARG[_l] for _l in range(L)]

_cache = {}
_last_run = {}


def _build(hb_zero=False):
    from contextlib import ExitStack

    import concourse.bacc as bacc
    import concourse.mybir as mybir
    import concourse.tile as tile

    dt = mybir.dt
    AF = mybir.ActivationFunctionType
    PM = mybir.MatmulPerfMode
    ALU = mybir.AluOpType
    f32, bf16, f8 = dt.float32, dt.bfloat16, dt.float8e4

    nc = bacc.Bacc("TRN2", target_bir_lowering=False, debug=False,
                   num_devices=NCORES)

    def din(name, shape, dty):
        return nc.dram_tensor(name, shape, dty, kind="ExternalInput").ap()

    xin_d = din("xin", [RD, W], bf16)
    # gated lhsT per layer [96, 64]: cols 0:32 tanh, 32:64 sigmoid(z/2)
    wgt_d = din("wgt", [96, L * 64], bf16)
    wdx_d = din("wdx", [128, L * 64], f8)         # dense lhsT, strip-placed
    wskp_d = din("wskp", [128, NG * 4 * 256], f8)  # skip lhsT DR per (G,m)
    wp2_d = din("wp2", [128, 2 * 2 * 256], f8)    # Wp2 lhsT DR per (p,qp)
    bgt_d = din("bgt", [128, L], f32)   # gated bias rows [bt, bs/2]x2
    hb_d = din("hb", [128, 4], f32)
    bp2c_d = din("bp2c", [128, 2], f32)
    sumw_d = din("sumw", [128, 1], bf16)
    nones_d = din("nones", [1, 128], bf16)
    out_d = nc.dram_tensor("out", [QD, V], f32, kind="ExternalOutput").ap()

    with tile.TileContext(nc) as tc, ExitStack() as top:
        wp = top.enter_context(tc.tile_pool(name="wp", bufs=1))

        def load(d, tag):
            t = wp.tile(list(d.shape), d.dtype, tag=tag, name=tag)
            nc.sync.dma_start(t[:], d[:])
            return t

        wgt = load(wgt_d, "wgt")
        wdx = load(wdx_d, "wdx")
        wskp = load(wskp_d, "wskp")
        wp2 = load(wp2_d, "wp2")
        bgt = load(bgt_d, "bgt")
        hb = load(hb_d, "hb")
        bp2c = load(bp2c_d, "bp2c")
        sumw = load(sumw_d, "sumw")
        nones = load(nones_d, "nones")

        xt = wp.tile([96, W], bf16, tag="xt", name="xt")
        ring = wp.tile([128, 2 * W], f8, tag="ring", name="ring")
        h = wp.tile([128, 4 * V], f8, tag="h", name="h")

        wdxv = wdx.rearrange("p (l two m) -> p l two m", two=2, m=RD)
        wskpv = wskp.rearrange("p (g two m) -> p g two m", two=2, m=128)
        wp2v = wp2.rearrange("p (q two m) -> p q two m", two=2, m=128)
        rv = ring.rearrange("p (two n) -> p two n", two=2)

        # input + initial replicas (layer 0 has d=1)
        nc.sync.dma_start(xt[0:RD, :], xin_d[:])
        d0 = DIL[0]
        for half in range(2):
            cs = half * (W // 2)
            ce = min(W - 2 * d0, (half + 1) * (W // 2))
            nc.sync.dma_start(xt[32:64, cs:ce], xt[0:RD, cs + d0:ce + d0])
            nc.sync.dma_start(xt[64:96, cs:ce],
                              xt[0:RD, cs + 2 * d0:ce + 2 * d0])

        with ExitStack() as lctx:
            pgp = lctx.enter_context(
                tc.tile_pool(name="pgp", bufs=2, space="PSUM"))
            pdp = lctx.enter_context(
                tc.tile_pool(name="pdp", bufs=2, space="PSUM"))
            tu = lctx.enter_context(tc.tile_pool(name="tu", bufs=3))

            for l in range(L):
                d = DIL[l]
                j = l % 8
                G = l // 8
                a = j % 4
                kt = j // 4
                nb = NBLK[l]
                st = START[l]

                # dense/res emitters, interleaved one quad behind gated
                dn = DIL[l + 1] if l < L - 1 else 1
                st2 = START[l + 1] if l < L - 1 else 0
                wd2 = (NBLK[l + 1] * 512) if l < L - 1 else 0
                nch = 4
                chw = -(-wd2 // nch) if wd2 else 0
                chunks = []
                for ch in range(nch if l < L - 1 else 0):
                    ccs = st2 - dn + ch * chw
                    cce = min(st2 - dn + (ch + 1) * chw, st2 + wd2)
                    if cce > ccs:
                        chunks.append((ccs, cce))
                chunk_i = [0]

                def dense_res_for_quad(qq):
                    if l >= L - 1:
                        return
                    for p in (2 * qq, 2 * qq + 1):
                        b0 = 2 * p
                        if b0 >= nb:
                            break
                        npr = 2 if b0 + 1 < nb else 1
                        cw = 512 * npr
                        c0 = st + b0 * 512
                        pd = pdp.tile([RD, 1024], f32, tag="pd", name="pd")
                        for s in range(npr):
                            cs = c0 + 512 * s
                            nc.tensor.matmul(
                                pd[:, 512 * s:512 * s + 512],
                                wdxv[RD * a:RD * a + RD, l, kt],
                                rv[RD * a:RD * a + RD, kt, cs:cs + 512],
                                start=True, stop=True,
                                tile_position=(RD * a, 0))
                        nc.vector.tensor_add(xt[0:RD, c0:c0 + cw],
                                             pd[:, 0:cw],
                                             xt[0:RD, c0:c0 + cw])
                        cov = st + min(nb, b0 + 2) * 512
                        while (chunk_i[0] < len(chunks)
                               and (chunks[chunk_i[0]][1] + 2 * dn <= cov
                                    or b0 + 2 >= nb)):
                            ccs, cce = chunks[chunk_i[0]]
                            nc.sync.dma_start(xt[32:64, ccs:cce],
                                              xt[0:RD, ccs + dn:cce + dn])
                            nc.sync.dma_start(
                                xt[64:96, ccs:cce],
                                xt[0:RD, ccs + 2 * dn:cce + 2 * dn])
                            chunk_i[0] += 1

                # ---- gated convs: quads of up to 4 blocks ----
                # pg quadrants: b0=(rows 0:64 [T;S], cols 0:512),
                # b1=(rows 0:64, cols 512:1024), b2=(rows 64:128 [S;T],
                # cols 0:512), b3=(rows 64:128, cols 512:1024).  sigmoid
                # then spans rows 32:96 contiguously for all 4 blocks.
                for q in range(-(-nb // 4)):
                    qn = min(4, nb - 4 * q)
                    topn = min(2, qn)
                    botn = qn - topn
                    ctop = st + 4 * q * 512
                    cbot = ctop + 1024
                    pg = pgp.tile([128, 1024], f32, tag="pg", name="pg")
                    for i in range(qn):
                        e = 0 if i < 2 else 1      # row half
                        col = (i % 2) * 512
                        cc = (ctop if e == 0 else cbot) + (i % 2) * 512
                        nc.tensor.matmul(
                            pg[64 * e:64 * e + 64, col:col + 512],
                            wgt[:, l * 64:l * 64 + 64],
                            xt[0:96, cc - d:cc - d + 512],
                            start=True, stop=True,
                            tile_position=(0, 64 * e))
                    tt = tu.tile([128, 1024], bf16, tag="tt", name="tt")
                    ss = tu.tile([128, 1024], bf16, tag="ss", name="ss")
                    nt = 512 * topn
                    nb_ = 512 * botn
                    rows = 128 if botn else 64
                    # single tanh: rows [t; tanh(z/2)] per pair; sigmoid is
                    # (1+s)/2, the x2 folded into halved dense/skip weights
                    nc.scalar.activation(tt[0:rows, 0:1024],
                                         pg[0:rows, 0:1024], AF.Tanh,
                                         bias=bgt[0:rows, l:l + 1])
                    nc.scalar.activation(ss[0:32, 0:nt], tt[32:64, 0:nt],
                                         AF.Identity, bias=1.0)
                    nc.gpsimd.tensor_mul(
                        rv[RD * a:RD * a + RD, kt, ctop:ctop + nt],
                        ss[0:32, 0:nt], tt[0:32, 0:nt])
                    if botn:
                        nc.scalar.activation(ss[64:96, 0:nb_],
                                             tt[96:128, 0:nb_],
                                             AF.Identity, bias=1.0)
                        nc.vector.tensor_mul(
                            rv[RD * a:RD * a + RD, kt, cbot:cbot + nb_],
                            ss[64:96, 0:nb_], tt[64:96, 0:nb_])
                    if q >= 1:
                        dense_res_for_quad(q - 1)
                dense_res_for_quad(-(-nb // 4) - 1)

                # ---- skip projection at group end (fp8 DR, K=256) ----
                if j == 7 or l == L - 1:
                    for cp in range(NB // 2):
                        c0 = VOFF + cp * 1024
                        for m in range(4):
                            pk = pdp.tile([128, 1024], f32, tag="pd",
                                          name="pk")
                            for s2 in range(2):
                                nc.tensor.matmul(
                                    pk[:, 512 * s2:512 * s2 + 512],
                                    wskpv[:, G * 4 + m],
                                    rv[:, :, c0 + 512 * s2:c0 + 512 * s2 + 512],
                                    start=True, stop=True,
                                    perf_mode=PM.DoubleRow)
                            hs = h[:, m * V + cp * 1024:
                                   m * V + cp * 1024 + 1024]
                            if G == 0:
                                nc.vector.tensor_copy(hs, pk[:])
                            else:
                                nc.vector.tensor_add(hs, pk[:], hs)

        # ---- post stage: relu -> Wp2 -> log_softmax -> out ----
        with ExitStack() as pctx:
            pop = pctx.enter_context(
                tc.tile_pool(name="pop", bufs=4, space="PSUM"))
            psp = pctx.enter_context(
                tc.tile_pool(name="psp", bufs=2, space="PSUM"))
            sp = pctx.enter_context(tc.tile_pool(name="sp", bufs=2))
            ALU = mybir.AluOpType
            for cb in range(NB):
                c0 = cb * 512
                rr = sp.tile([128, 4 * 512], f8, tag="rr", name="rr")
                rrv = rr.rearrange("p (q n) -> p q n", q=4)
                if hb_zero:
                    hv = h.rearrange("p (q n) -> p q n", q=4)
                    nc.scalar.activation(rrv[:, :, :], hv[:, :, c0:c0 + 512],
                                         AF.Relu)
                else:
                    for m in range(4):
                        nc.scalar.activation(
                            rr[:, m * 512:(m + 1) * 512],
                            h[:, m * V + c0:m * V + c0 + 512],
                            AF.Relu, bias=hb[:, m:m + 1])
                ee = sp.tile([128, 1024], bf16, tag="ee", name="ee")
                pos = []
                for pp_ in range(2):
                    po = pop.tile([128, 512], f32, tag="po", name="po")
                    nc.tensor.matmul(po[:], wp2v[:, 2 * pp_], rrv[:, 0:2, :],
                                     start=True, stop=False,
                                     perf_mode=PM.DoubleRow)
                    nc.tensor.matmul(po[:], wp2v[:, 2 * pp_ + 1],
                                     rrv[:, 2:4, :],
                                     start=False, stop=True,
                                     perf_mode=PM.DoubleRow)
                    nc.scalar.activation(ee[:, 512 * pp_:512 * pp_ + 512],
                                         po[:], AF.Exp,
                                         bias=bp2c[:, pp_:pp_ + 1])
                    pos.append(po)
                ps_ = psp.tile([1, 512], f32, tag="ps", name="ps")
                nc.tensor.matmul(ps_[0:1, :], sumw[:], ee[:, 0:512],
                                 start=True, stop=False)
                nc.tensor.matmul(ps_[0:1, :], sumw[:], ee[:, 512:1024],
                                 start=False, stop=True)
                lss = sp.tile([1, 512], bf16, tag="ls", name="ls")
                nc.scalar.activation(lss[:], ps_[0:1, :], AF.Ln)
                oo = sp.tile([128, 1024], f32, tag="oo", name="oo")
                for pp_ in range(2):
                    # accumulate -ln(sumexp) into the Wp2 psum, then one
                    # Identity act adds bp2 and writes the final output
                    nc.tensor.matmul(pos[pp_][:], nones[:], lss[:],
                                     start=False, stop=True,
                                     skip_group_check=True)
                    nc.scalar.activation(oo[:, 512 * pp_:512 * pp_ + 512],
                                         pos[pp_][:], AF.Identity,
                                         bias=bp2c[:, pp_:pp_ + 1])
                    nc.sync.dma_start(
                        out_d[pp_ * 128:(pp_ + 1) * 128, c0:c0 + 512],
                        oo[:, 512 * pp_:512 * pp_ + 512])

    nc.compile()
    return nc


def _prep_host(inputs):
    """Host-side fp32 preprocessing: initial conv, bias folding, packing."""
    x = np.asarray(inputs["x"], np.float32)
    Wc = np.asarray(inputs["Wc"], np.float32)
    bc = np.asarray(inputs["bc"], np.float32)
    Wt = np.asarray(inputs["Wt"], np.float32)
    bt = np.asarray(inputs["bt"], np.float32)
    Ws = np.asarray(inputs["Ws"], np.float32)
    bs = np.asarray(inputs["bs"], np.float32)
    Wskip = np.asarray(inputs["Wskip"], np.float32)
    bskip = np.asarray(inputs["bskip"], np.float32)
    Wdense = np.asarray(inputs["Wdense"], np.float32)
    bdense = np.asarray(inputs["bdense"], np.float32)
    Wp1 = np.asarray(inputs["Wp1"], np.float32)
    bp1 = np.asarray(inputs["bp1"], np.float32)
    Wp2 = np.asarray(inputs["Wp2"], np.float32)
    bp2 = np.asarray(inputs["bp2"], np.float32)

    # initial conv (1 -> 32, k=3, pad=1), exact fp32 on host
    x0 = x[0, 0]
    xp = np.pad(x0, (1, 1))
    x1 = (Wc[:, 0, 0:1] * xp[None, 0:T]
          + Wc[:, 0, 1:2] * xp[None, 1:T + 1]
          + Wc[:, 0, 2:3] * xp[None, 2:T + 2]) + bc[:, None]
    xg = np.pad(x1, ((0, 0), (VOFF, W)))  # generous right pad

    xin = np.empty((NCORES, RD, W), BF16)
    for c in range(NCORES):
        s = c * V
        xin[c] = xg[:, s:s + W].astype(BF16)

    # cumulative dense-bias folding
    B = np.zeros((L, RD), np.float32)
    for l in range(1, L):
        B[l] = B[l - 1] + bdense[l - 1]

    wgt = np.zeros((96, L * 64), np.float32)
    btl = np.zeros((L, RD), np.float32)
    bsl = np.zeros((L, RD), np.float32)
    for l in range(L):
        for k in range(3):
            wgt[32 * k:32 * (k + 1), l * 64:l * 64 + 32] = Wt[l, :, :, k].T
            wgt[32 * k:32 * (k + 1),
                l * 64 + 32:l * 64 + 64] = Ws[l, :, :, k].T * 0.5
        btl[l] = bt[l] + Wt[l].sum(axis=2) @ B[l]
        bsl[l] = (bs[l] + Ws[l].sum(axis=2) @ B[l]) * 0.5

    wdx = np.zeros((128, L * 64), np.float32)
    for l in range(L - 1):
        a, kt = (l % 8) % 4, (l % 8) // 4
        wdx[32 * a:32 * a + 32,
            l * 64 + kt * 32:l * 64 + kt * 32 + 32] = Wdense[l, :, :, 0].T * 0.5

    W1s = np.einsum("ab,lbc->lac", Wp1[:, :, 0], Wskip[:, :, :, 0]) * 0.5
    wskp = np.zeros((128, NG * 4 * 256), np.float32)
    for G in range(NG):
        for m in range(4):
            base = (G * 4 + m) * 256
            for j in range(8):
                l = G * 8 + j
                if l < L:
                    a, kt = j % 4, j // 4
                    wskp[32 * a:32 * (a + 1),
                         base + kt * 128:base + kt * 128 + 128] = \
                        W1s[l, 128 * m:128 * (m + 1), :].T

    wp2 = np.zeros((128, 2 * 2 * 256), np.float32)
    for pp_ in range(2):
        for qp in range(2):
            base = (2 * pp_ + qp) * 256
            for half in range(2):
                q = qp * 2 + half
                wp2[:, base + half * 128:base + half * 128 + 128] = \
                    Wp2[128 * pp_:128 * (pp_ + 1),
                        128 * q:128 * (q + 1), 0].T

    hbias = Wp1[:, :, 0] @ bskip.sum(axis=0) + bp1
    hbm = hbias.reshape(4, 128).T.copy()

    shared = {
        "wgt": wgt.astype(BF16),
        "wdx": wdx.astype(F8),
        "wskp": wskp.astype(F8),
        "wp2": wp2.astype(F8),
        "bgt": np.ascontiguousarray(np.concatenate(
            [btl.T, bsl.T, btl.T, bsl.T], axis=0).astype(np.float32)),
        "hb": np.ascontiguousarray(hbm.astype(np.float32)),
        "bp2c": np.ascontiguousarray(bp2.reshape(2, 128).T.astype(np.float32)),
        "sumw": np.ones((128, 1), BF16),
        "nones": np.full((1, 128), -1.0, BF16),
    }
    return xin, shared


def kernel(**inputs):
    from concourse.bass_utils import run_bass_kernel_spmd

    xin, shared = _prep_host(inputs)
    hb_zero = bool(np.all(shared["hb"] == 0.0))
    key = ("nc", hb_zero)
    if key not in _cache:
        _cache[key] = _build(hb_zero)
    nc = _cache[key]

    in_maps = [dict(shared, xin=np.ascontiguousarray(xin[c]))
               for c in range(NCORES)]
    res = run_bass_kernel_spmd(nc, in_maps, core_ids=list(range(NCORES)))

    _last_run["nc"] = nc
    _last_run["in_maps"] = in_maps

    out = np.empty((1, QD, T), np.float32)
    for c in range(NCORES):
        out[0, :, c * V:(c + 1) * V] = res.results[c]["out"]
    return out
